# revision 6
# baseline (speedup 1.0000x reference)
"""CIN (Compressed Interaction Network) kernel for Trainium2, 8 NeuronCores.

Problem: x (2048, 39, 16) f32; 3 CIN layers with W_i (200, 39, prev):
    z[b,o,d] = sum_{f,g} W[o,f,g] * x0[b,f,d] * h[b,g,d] + bias[o]
    h' = relu(z);  output = sum_d concat([h1,h2,h3], ch) -> (2048, 600)

Strategy (data-parallel over batch, 8 cores, 256 batch rows each):
  Per core, columns n = (b_local, d), N = 256*16 = 4096, in 16 n-tiles of 256
  (two 128-column windows each).  Matmuls run in the z^T orientation:
  out psum [128 n, 200 o] accumulates over the contraction (f, g) --
  streaming all 200 output channels in one pass instead of two 128/72
  splits.  lhsT (stationary) = V slices [g, n-window]; rhs (moving) =
  weight slices [g, 200].  V_f = h (.) bcast(x0[f]):
    f 0..7   built in fp8 directly on the GPSIMD/Pool engine
    f 8..15  built fp16 on Vector, converted to fp8 by the Scalar engine
    f 16..38 built fp16 on Vector (widened, 4-8 f's per instruction)
  f 0..15 are consumed by fp8e4m3 DoubleRow matmuls (two f's contracted
  per instruction at 0.5 cycles/row).  All weights are pre-scaled by 64
  so fp8 stays in e4m3's normal range; the relu epilogue on the Scalar
  engine rescales by 1/64.  Bias enters as a K=1 ones-row matmul.
  h^T [n, 200] is transposed back to [g, n] tiles by the PE (identity
  transpose) for the next layer's V build; d-sums run on the Vector
  engine.  Tiles are emitted pairwise-interleaved so the PE always has
  an independent tile's matmuls (keeps the p-state clock ramped).
"""
import numpy as np

import concourse.bacc as bacc
import concourse.mybir as mybir
import concourse.tile as tile
from concourse.bass_utils import run_bass_kernel_spmd

B, F0, D = 2048, 39, 16
C = 200                      # cross size per layer
NCORES = 8
BC = B // NCORES             # 256 batch rows per core
N = BC * D                   # 4096 columns per core
NT = 256                     # n-tile width
T = N // NT                  # 16 n-tiles
BT = NT // D                 # 16 batch rows per n-tile
NW = NT // 128               # 2 matmul windows per tile
KF0 = (F0 * F0 + 127) // 128  # 12 flat L0 K-chunks (zero-padded)
NF8 = 20                     # f's 0..NF8-1 in fp8 DoubleRow (pairs)
NFP = 8                      # of those, f's 0..NFP-1 built on Pool
NP8 = NF8 // 2               # 8 pairs
NF16 = F0 - NF8              # 23 f's in fp16 matmuls
NV16 = F0 - NFP              # 31 f's built in fp16 on Vector (f 8..38)
GA, GB = 128, C - 128        # g-chunks (h partition split 128 + 72)
SCALE = 64.0                 # weight pre-scale (power of 2)
F16 = mybir.dt.float16
F8 = mybir.dt.float8e4
F32 = mybir.dt.float32


def _build_nc():
    nc = bacc.Bacc(None, target_bir_lowering=False)
    mult = mybir.AluOpType.mult
    relu = mybir.ActivationFunctionType.Relu

    x0_d = nc.dram_tensor("x0", [F0, N], F16, kind="ExternalInput")
    v0_d = nc.dram_tensor("v0", [KF0 * 128, N], F16, kind="ExternalInput")
    w0_d = nc.dram_tensor("w0", [128, KF0 * C], F16, kind="ExternalInput")
    w1a_d = nc.dram_tensor("w1a", [GA, NF16 * C], F16, kind="ExternalInput")
    w1b_d = nc.dram_tensor("w1b", [GB, NF16 * C], F16, kind="ExternalInput")
    w2a_d = nc.dram_tensor("w2a", [GA, NF16 * C], F16, kind="ExternalInput")
    w2b_d = nc.dram_tensor("w2b", [GB, NF16 * C], F16, kind="ExternalInput")
    w18a_d = nc.dram_tensor("w18a", [GA, NF8 * C], F8, kind="ExternalInput")
    w18b_d = nc.dram_tensor("w18b", [GB, NF8 * C], F8, kind="ExternalInput")
    w28a_d = nc.dram_tensor("w28a", [GA, NF8 * C], F8, kind="ExternalInput")
    w28b_d = nc.dram_tensor("w28b", [GB, NF8 * C], F8, kind="ExternalInput")
    brow_d = nc.dram_tensor("brow", [1, 3 * C], F16, kind="ExternalInput")
    ones_d = nc.dram_tensor("ones1", [1, 128], F16, kind="ExternalInput")
    id_d = nc.dram_tensor("ident", [128, 128], F16, kind="ExternalInput")
    out_d = nc.dram_tensor("out3", [3, C, BC], F32, kind="ExternalOutput")

    with tile.TileContext(nc) as tc:
        with (
            tc.tile_pool(name="wp", bufs=1) as wp,
            tc.tile_pool(name="bc", bufs=2) as bcp,
            tc.tile_pool(name="hp", bufs=2) as hp,
            tc.tile_pool(name="ht", bufs=4) as htp,
            tc.tile_pool(name="va", bufs=2) as vap,
            tc.tile_pool(name="ps", bufs=3, space="PSUM") as ps,
            tc.tile_pool(name="pt", bufs=2, space="PSUM") as pt,
        ):
            # --- static state -------------------------------------------------
            w0 = wp.tile([128, KF0 * C], F16)
            nc.sync.dma_start(out=w0[:], in_=w0_d[:])
            brow = wp.tile([1, 3 * C], F16)
            nc.sync.dma_start(out=brow[:], in_=brow_d[:])
            ones1 = wp.tile([1, 128], F16)
            nc.sync.dma_start(out=ones1[:], in_=ones_d[:])
            ident = wp.tile([128, 128], F16)
            nc.sync.dma_start(out=ident[:], in_=id_d[:])
            outs = []
            for l in range(3):
                oa = wp.tile([GA, BC], F32, tag=f"o{l}a")
                ob = wp.tile([GB, BC], F32, tag=f"o{l}b")
                outs.append((oa, ob))

            def emit_v0(t):
                v0t = bcp.tile([128, KF0 * NT], F16, tag="v0t")
                src = (v0_d[:].rearrange("(c p) n -> p c n", p=128)
                       [:, :, t * NT:(t + 1) * NT])
                for c0 in range(0, KF0, 6):
                    c1 = min(c0 + 6, KF0)
                    nc.sync.dma_start(
                        out=v0t[:, c0 * NT:c1 * NT]
                        .rearrange("p (c n) -> p c n", n=NT),
                        in_=src[:, c0:c1, :])
                return v0t

            def emit_xb(t, fchunk=13):
                xb = bcp.tile([128, F0 * NT], F16, tag="xb")
                for f0 in range(0, F0, fchunk):
                    f1 = min(f0 + fchunk, F0)
                    src = (x0_d[f0:f1, t * NT:(t + 1) * NT]
                           .unsqueeze(0).broadcast_to((128, f1 - f0, NT)))
                    nc.sync.dma_start(
                        out=xb[:, f0 * NT:f1 * NT]
                        .rearrange("p (f n) -> p f n", n=NT), in_=src)
                return xb

            def emit_build(xb, ha, hb):
                # V tiles for one (tile, layer):
                #   va8/vb8 [*, NF8*NT] f8: f 0..NFP-1 Pool-direct,
                #     f NFP..NF8-1 Act-converted from va
                #   va/vb [*, NV16*NT] f16: f NFP..38 on Vector
                va = vap.tile([GA, NV16 * NT], F16, tag="va")
                vb = vap.tile([GB, NV16 * NT], F16, tag="vb")
                va8 = vap.tile([GA, NF8 * NT], F8, tag="va8")
                vb8 = vap.tile([GB, NF8 * NT], F8, tag="vb8")
                fs = slice(0, NFP * NT)
                nc.gpsimd.tensor_tensor(
                    out=va8[:, fs].rearrange("p (f n) -> p f n", n=NT),
                    in0=ha[:].unsqueeze(1).broadcast_to((GA, NFP, NT)),
                    in1=xb[0:GA, fs].rearrange("p (f n) -> p f n", n=NT),
                    op=mult)
                nc.gpsimd.tensor_tensor(
                    out=vb8[:, fs].rearrange("p (f n) -> p f n", n=NT),
                    in0=hb[:].unsqueeze(1).broadcast_to((GB, NFP, NT)),
                    in1=xb[0:GB, fs].rearrange("p (f n) -> p f n", n=NT),
                    op=mult)
                for j0 in range(0, NV16, 8):
                    j1 = min(j0 + 8, NV16)
                    w = j1 - j0
                    fs = slice((NFP + j0) * NT, (NFP + j1) * NT)
                    nc.vector.tensor_tensor(
                        out=va[:, j0 * NT:j1 * NT]
                        .rearrange("p (f n) -> p f n", n=NT),
                        in0=ha[:].unsqueeze(1).broadcast_to((GA, w, NT)),
                        in1=xb[0:GA, fs].rearrange("p (f n) -> p f n", n=NT),
                        op=mult)
                    nc.vector.tensor_tensor(
                        out=vb[:, j0 * NT:j1 * NT]
                        .rearrange("p (f n) -> p f n", n=NT),
                        in0=hb[:].unsqueeze(1).broadcast_to((GB, w, NT)),
                        in1=xb[0:GB, fs].rearrange("p (f n) -> p f n", n=NT),
                        op=mult)
                # Act converts va positions 0..NF8-NFP-1 (f NFP..NF8-1)
                cs = slice(0, (NF8 - NFP) * NT)
                c8 = slice(NFP * NT, NF8 * NT)
                nc.scalar.copy(out=va8[:, c8], in_=va[:, cs])
                nc.scalar.copy(out=vb8[:, c8], in_=vb[:, cs])
                return va, vb, va8, vb8

            def emit_l0_win(v0t, w, l):
                # z^T psum [128 n, 200] over 12 flat K-chunks + bias row
                pz = ps.tile([128, C], F32, tag="pz")
                v3 = v0t[:].rearrange("p (c n) -> p c n", n=NT)
                for c in range(KF0):
                    nc.tensor.matmul(pz[:], v3[:, c, w * 128:(w + 1) * 128],
                                     w0[:, c * C:(c + 1) * C],
                                     start=(c == 0), stop=False)
                nc.tensor.matmul(pz[:], ones1[:],
                                 brow[:, l * C:(l + 1) * C],
                                 start=False, stop=True)
                return pz

            def emit_l12_win(vs, wa, wb, w8a, w8b, w, l):
                va, vb, va8, vb8 = vs
                pz = ps.tile([128, C], F32, tag="pz")
                ws = slice(w * 128, (w + 1) * 128)
                v3a = va[:].rearrange("p (f n) -> p f n", n=NT)
                v3b = vb[:].rearrange("p (f n) -> p f n", n=NT)
                for j in range(NF16):
                    # fp16 f = NF8 + j lives at va position (NF8 - NFP) + j
                    p = (NF8 - NFP) + j
                    nc.tensor.matmul(pz[:], v3a[:, p, ws],
                                     wa[:, j * C:(j + 1) * C],
                                     start=(j == 0), stop=False)
                    nc.tensor.matmul(pz[:], v3b[:, p, ws],
                                     wb[:, j * C:(j + 1) * C],
                                     start=False, stop=False)
                p3a = va8[:].rearrange("p (f n) -> p f n", n=NT)
                p3b = vb8[:].rearrange("p (f n) -> p f n", n=NT)
                w4a = w8a[:].rearrange("p (j t o) -> p j t o", t=2, o=C)
                w4b = w8b[:].rearrange("p (j t o) -> p j t o", t=2, o=C)
                for j in range(NP8):
                    nc.tensor.matmul(pz[:], p3a[:, 2 * j:2 * j + 2, ws],
                                     w4a[:, j], start=False, stop=False,
                                     perf_mode=mybir.MatmulPerfMode.DoubleRow)
                    nc.tensor.matmul(pz[:], p3b[:, 2 * j:2 * j + 2, ws],
                                     w4b[:, j], start=False, stop=False,
                                     perf_mode=mybir.MatmulPerfMode.DoubleRow)
                nc.tensor.matmul(pz[:], ones1[:],
                                 brow[:, l * C:(l + 1) * C],
                                 start=False, stop=True)
                return pz

            def emit_epi(pz, w, ha, hb):
                # relu (+1/64 rescale) -> h^T [128, 200]; transpose to h tiles
                hT = htp.tile([128, C], F16, tag="hT")
                nc.scalar.activation(hT[:], pz[:], relu, scale=1.0 / SCALE)
                pa = pt.tile([128, 128], F16, tag="pta")
                pb = pt.tile([GB, 128], F16, tag="ptb")
                nc.tensor.transpose(pa[:], hT[:, 0:GA], ident[:])
                nc.tensor.transpose(pb[:], hT[:, GA:C], ident[:])
                ws = slice(w * 128, (w + 1) * 128)
                nc.scalar.copy(out=ha[:, ws], in_=pa[:])
                nc.scalar.copy(out=hb[:, ws], in_=pb[:])

            def emit_reduce(t, l, ha, hb):
                oa, ob = outs[l]
                bs = slice(t * BT, (t + 1) * BT)
                nc.vector.tensor_reduce(
                    out=oa[:, bs], in_=ha[:].rearrange("p (b d) -> p b d", d=D),
                    axis=mybir.AxisListType.X, op=mybir.AluOpType.add)
                nc.vector.tensor_reduce(
                    out=ob[:, bs], in_=hb[:].rearrange("p (b d) -> p b d", d=D),
                    axis=mybir.AxisListType.X, op=mybir.AluOpType.add)

            # --- pipeline: pairwise-interleaved n-tiles ----------------------
            v00, v01 = emit_v0(0), emit_v0(1)
            xb0, xb1 = emit_xb(0), emit_xb(1)
            w1a = wp.tile([GA, NF16 * C], F16)
            nc.sync.dma_start(out=w1a[:], in_=w1a_d[:])
            w1b = wp.tile([GB, NF16 * C], F16)
            nc.sync.dma_start(out=w1b[:], in_=w1b_d[:])
            w18a = wp.tile([GA, NF8 * C], F8)
            nc.sync.dma_start(out=w18a[:], in_=w18a_d[:])
            w18b = wp.tile([GB, NF8 * C], F8)
            nc.sync.dma_start(out=w18b[:], in_=w18b_d[:])
            w2a = wp.tile([GA, NF16 * C], F16)
            nc.sync.dma_start(out=w2a[:], in_=w2a_d[:])
            w2b = wp.tile([GB, NF16 * C], F16)
            nc.sync.dma_start(out=w2b[:], in_=w2b_d[:])
            w28a = wp.tile([GA, NF8 * C], F8)
            nc.sync.dma_start(out=w28a[:], in_=w28a_d[:])
            w28b = wp.tile([GB, NF8 * C], F8)
            nc.sync.dma_start(out=w28b[:], in_=w28b_d[:])

            for tp in range(0, T, 2):
                t0, t1 = tp, tp + 1
                hs = []
                for k in range(2):
                    row = []
                    for l in range(3):
                        hta = hp.tile([GA, NT], F16, tag=f"h{l}a{k}",
                                      name=f"h{l}a{k}_{tp}")
                        htb = hp.tile([GB, NT], F16, tag=f"h{l}b{k}",
                                      name=f"h{l}b{k}_{tp}")
                        row.append((hta, htb))
                    hs.append(tuple(row))
                v0s, xbs = (v00, v01), (xb0, xb1)
                # L0 both tiles
                for k, t in ((0, t0), (1, t1)):
                    for w in range(NW):
                        pz = emit_l0_win(v0s[k], w, 0)
                        emit_epi(pz, w, *hs[k][0])
                # build V1 + L1 both tiles
                vss = []
                for k, t in ((0, t0), (1, t1)):
                    emit_reduce(t, 0, *hs[k][0])
                    vss.append(emit_build(xbs[k], *hs[k][0]))
                for k, t in ((0, t0), (1, t1)):
                    for w in range(NW):
                        pz = emit_l12_win(vss[k], w1a, w1b, w18a, w18b, w, 1)
                        emit_epi(pz, w, *hs[k][1])
                # build V2 + L2 both tiles
                vss = []
                for k, t in ((0, t0), (1, t1)):
                    emit_reduce(t, 1, *hs[k][1])
                    vss.append(emit_build(xbs[k], *hs[k][1]))
                for k, t in ((0, t0), (1, t1)):
                    for w in range(NW):
                        pz = emit_l12_win(vss[k], w2a, w2b, w28a, w28b, w, 2)
                        emit_epi(pz, w, *hs[k][2])
                for k, t in ((0, t0), (1, t1)):
                    emit_reduce(t, 2, *hs[k][2])
                if tp + 2 < T:
                    v00, v01 = emit_v0(tp + 2), emit_v0(tp + 3)
                    xb0, xb1 = emit_xb(tp + 2), emit_xb(tp + 3)

            for l in range(3):
                oa, ob = outs[l]
                nc.sync.dma_start(out=out_d[l, 0:GA, :], in_=oa[:])
                nc.sync.dma_start(out=out_d[l, GA:C, :], in_=ob[:])

    nc.compile()
    return nc


_NC_CACHE = None


def _get_nc():
    global _NC_CACHE
    if _NC_CACHE is None:
        _NC_CACHE = _build_nc()
    return _NC_CACHE


def _q8(x):
    import ml_dtypes
    return np.asarray(x, np.float32).astype(ml_dtypes.float8_e4m3fn)


def _prep_weights(W0, b0, W1, b1, W2, b2):
    # L0 rhs layout: w0[p, c*C+o] = SCALE*W0[o, f, g], flat k=128c+p=f*39+g
    W0 = np.asarray(W0, np.float32)
    w0f = np.zeros((KF0 * 128, C), np.float32)
    w0f[0:F0 * F0] = W0.reshape(C, F0 * F0).T * SCALE
    w0 = np.ascontiguousarray(
        w0f.reshape(KF0, 128, C).transpose(1, 0, 2).reshape(128, KF0 * C)
    ).astype(np.float16)

    def lay(W):
        # Wt[g, f, o] = SCALE*W[o, f, g]
        Wt = np.asarray(W, np.float32).transpose(2, 1, 0) * SCALE
        wa = np.ascontiguousarray(Wt[0:GA, NF8:].reshape(GA, NF16 * C)
                                  ).astype(np.float16)
        wb = np.ascontiguousarray(Wt[GA:C, NF8:].reshape(GB, NF16 * C)
                                  ).astype(np.float16)
        w8a = _q8(np.ascontiguousarray(Wt[0:GA, 0:NF8].reshape(GA, NF8 * C)))
        w8b = _q8(np.ascontiguousarray(Wt[GA:C, 0:NF8].reshape(GB, NF8 * C)))
        return wa, wb, w8a, w8b

    w1a, w1b, w18a, w18b = lay(W1)
    w2a, w2b, w28a, w28b = lay(W2)
    brow = np.zeros((1, 3 * C), np.float16)
    for l, b in enumerate((b0, b1, b2)):
        brow[0, l * C:(l + 1) * C] = (np.asarray(b, np.float32) * SCALE
                                      ).astype(np.float16)
    return {
        "w0": w0, "w1a": w1a, "w1b": w1b, "w2a": w2a, "w2b": w2b,
        "w18a": w18a, "w18b": w18b, "w28a": w28a, "w28b": w28b,
        "brow": brow,
        "ones1": np.ones((1, 128), np.float16),
        "ident": np.eye(128, dtype=np.float16),
    }


def kernel(x, W0, b0, W1, b1, W2, b2):
    x = np.asarray(x)
    assert x.shape == (B, F0, D), x.shape
    nc = _get_nc()
    shared = _prep_weights(W0, b0, W1, b1, W2, b2)

    in_maps = []
    for c in range(NCORES):
        xc = x[c * BC:(c + 1) * BC]                      # [256, 39, 16]
        x0c = np.ascontiguousarray(
            xc.transpose(1, 0, 2).reshape(F0, N)).astype(np.float16)
        x0f32 = x0c.astype(np.float32)
        v0 = np.zeros((KF0 * 128, N), np.float16)
        v0[0:F0 * F0] = (x0f32[:, None, :] * x0f32[None, :, :]
                         ).reshape(F0 * F0, N).astype(np.float16)
        in_maps.append({"x0": x0c, "v0": v0, **shared})

    res = run_bass_kernel_spmd(nc, in_maps, list(range(NCORES)))

    out = np.empty((B, 3 * C), dtype=np.float32)
    for c in range(NCORES):
        o3 = res.results[c]["out3"]                      # [3, 200, 256]
        out[c * BC:(c + 1) * BC] = o3.transpose(2, 0, 1).reshape(BC, 3 * C)
    return out
